# revision 10
# baseline (speedup 1.0000x reference)
"""Trainium2 Bass kernel for nn_ApproxCompressor (v5).

Reference (per sample n):
    alpha = sigmoid(z_alpha); h[k] = (1-alpha)*alpha^k (k<16384)
    env   = causal_conv(mean_c x^2, h); LG = log(env + 1e-5)
    quadratic-knee gain; out = gain * x.

v5 strategy (8 cores x 4 samples, pure data parallel):
  * Layout: sample s owns partitions [32s, 32s+32); each partition holds a
    contiguous 4096-sample run, stored as 4 chunks of [ch0 1024 | ch1 1024]
    (4KB contiguous per partition per chunk) for chunk-granular DMA.
  * 16K-tap FIR == one-pole IIR via DVE tensor_tensor_scan per chunk,
    chained via `initial`; cross-partition carries fixed by a block-diagonal
    decay matmul (PE) + power-table stt on the first JF cols of chunk 0.
    Chunk 0 is split head[0:JF] (waits on fix) / tail[JF:] (early).
  * Exact knee, 0 relu + 0 stt:  v = ln(e^{W-T}*(s*y + eps)) = LG - T + W
    (shift folded into ln scale/bias);  C = clamp(v,0,2W) and Z' = 2v-2W
    are single dual-op tensor_scalars (DVE 2x_2p); Z = max(Z',C), Q = C*Z
    bf16 tensor_tensors; gain = exp(c4w*Q) via ACT scale.
  * Knee ops for chunks 1..3 run on the idle Pool engine (nc.gpsimd);
    chunk 0 stays on DVE (end-of-kernel slack).
  * Params packed in one aux tensor [128, 8+128+256], loaded via Pool SWDGE.
"""

import os
import sys

import numpy as np


def _import_concourse():
    try:
        import concourse.bass  # noqa: F401
    except ImportError:
        for p in ("/opt/trn_rl_repo", "/root/.axon_site/_ro/trn_rl_repo"):
            if os.path.isdir(p) and p not in sys.path:
                sys.path.insert(0, p)
        import concourse.bass  # noqa: F401


_import_concourse()

import ml_dtypes  # noqa: E402
import concourse.bass as bass  # noqa: E402
import concourse.tile as tile  # noqa: E402
from concourse import bacc, mybir  # noqa: E402

N, C, L = 32, 2, 131072
NCORES = 8
NLOC = N // NCORES  # 4 samples/core
P = 128
SPP = P // NLOC  # 32 partitions/sample
FCH = L // SPP  # 4096 samples per partition row
NCH = 4  # chunks
W_CH = FCH // NCH  # 1024
JF = 256
EPS = 1e-5
K_FIR = 16384
ROW = C * FCH  # 8192 elems per device-layout row
CW = C * W_CH  # 2048 elems per chunk row

F32 = mybir.dt.float32
BF16 = mybir.dt.bfloat16

# aux columns: 0 alpha | 1 lnscale2 | 2 eps2 | 3 w2 | 4 c4w ; 8:136 tri ; 136:392 pw
A_ALPHA, A_LNS, A_EPS, A_W2, A_C4W = 0, 1, 2, 3, 4
A_TRI, A_PW = 8, 136
NAUX = A_PW + JF

ACT_SET_ID = 6
# probe: dual-op tensor_scalars (C, Z') on Pool for chunks 1-3
TS_POOL = (False, True, True, True)
# probe: scan engine per chunk (Pool lowering of TensorScalarPtr-scan)
SCAN_POOL = (False, False, False, False)

TRACE_RESULT = {}


def _bcast(col_ap, n):
    return bass.AP(col_ap.tensor, col_ap.offset, [list(col_ap.ap[0]), [0, n]])


def _view3(ap2, c, w):
    """[p, w] slice -> [p, c, w] with stride-0 mid dim if c>1 over gain."""
    return bass.AP(ap2.tensor, ap2.offset, [list(ap2.ap[0]), [0, c], [1, w]])


def build_nc():
    AF = mybir.ActivationFunctionType
    OP = mybir.AluOpType

    nc = bacc.Bacc("TRN2", target_bir_lowering=False, num_devices=NCORES)
    xd_ext = nc.declare_dram_parameter("xd", [P, ROW], BF16, isOutput=False)
    aux_ext = nc.declare_dram_parameter("aux", [P, NAUX], F32, isOutput=False)
    od_ext = nc.declare_dram_parameter("od", [P, ROW], BF16, isOutput=True)

    with tile.TileContext(nc) as tc:
        atl = mybir.InstLoadActFuncSet(
            name=nc.get_next_instruction_name(), ins=[], outs=[],
            act_func_set_id=ACT_SET_ID,
        )
        nc.scalar.add_instruction(atl)
        with (
            tc.tile_pool(name="pc", bufs=1) as pc,
            tc.tile_pool(name="pin", bufs=NCH) as pin,
            tc.tile_pool(name="po", bufs=NCH) as po,
            tc.tile_pool(name="py", bufs=NCH) as py,
            tc.tile_pool(name="psq", bufs=3) as psq,
            tc.tile_pool(name="pD", bufs=NCH) as pD,
            tc.tile_pool(name="pv", bufs=NCH) as pv,
            tc.tile_pool(name="pcz", bufs=6) as pcz,
            tc.tile_pool(name="pg", bufs=NCH) as pg,
            tc.tile_pool(name="pps", bufs=2, space=bass.MemorySpace.PSUM) as pps,
        ):
            aux = pc.tile([P, NAUX], F32, tag="aux")
            nc.gpsimd.dma_start(out=aux[:], in_=aux_ext[:])

            a_col = aux[:, A_ALPHA : A_ALPHA + 1]
            lns_col = aux[:, A_LNS : A_LNS + 1]
            eps_col = aux[:, A_EPS : A_EPS + 1]
            w2_col = aux[:, A_W2 : A_W2 + 1]
            c4w_col = aux[:, A_C4W : A_C4W + 1]
            tri_ap = aux[:, A_TRI : A_TRI + P]
            pw_ap = aux[:, A_PW : A_PW + JF]

            # ---- input: per-chunk tiles, 2 partition-half DMA calls each ----
            xt = []
            for k in range(NCH):
                xk = pin.tile([P, CW], BF16, tag="xk")
                nc.sync.dma_start(
                    out=xk[0:64, :], in_=xd_ext[0:64, k * CW : (k + 1) * CW]
                )
                nc.scalar.dma_start(
                    out=xk[64:128, :], in_=xd_ext[64:128, k * CW : (k + 1) * CW]
                )
                xt.append(xk)

            # ---- energy (ACT square + DVE add) + chained scans ----------
            y1 = []
            for k in range(NCH):
                sq = psq.tile([P, CW], BF16, tag="sq")
                nc.scalar.activation(sq[:], xt[k][:], AF.Square)
                D = pD.tile([P, W_CH], BF16, tag="D")
                nc.vector.tensor_tensor(
                    D[:], sq[:, 0:W_CH], sq[:, W_CH:CW], OP.add
                )
                yk = py.tile([P, W_CH], F32, tag="y1")
                init = 0.0 if k == 0 else y1[k - 1][:, W_CH - 1 : W_CH]
                seng = nc.gpsimd if SCAN_POOL[k] else nc.vector
                seng.tensor_tensor_scan(
                    yk[:], _bcast(a_col, W_CH), D[:], init, OP.mult, OP.add
                )
                y1.append(yk)

            # ---- knee pipeline -------------------------------------------
            vt = [None] * NCH
            gaint = [None] * NCH

            def knee(k, lo, hi, ts_eng):
                """v[lo:hi] -> gain[lo:hi] for chunk k; ts ops on ts_eng."""
                v = vt[k][:, lo:hi]
                Ct = pcz.tile([P, W_CH], BF16, tag="C", name=f"C{k}")
                Zp = pcz.tile([P, W_CH], BF16, tag="Zp", name=f"Zp{k}")
                ts_eng.tensor_scalar(Ct[:, lo:hi], v, w2_col, 0.0, OP.min, OP.max)
                ts_eng.tensor_scalar(
                    Zp[:, lo:hi], v, 2.0, w2_col, OP.mult, OP.subtract
                )
                nc.vector.tensor_tensor(Zp[:, lo:hi], Zp[:, lo:hi], Ct[:, lo:hi], OP.max)
                nc.vector.tensor_tensor(Ct[:, lo:hi], Ct[:, lo:hi], Zp[:, lo:hi], OP.mult)
                nc.scalar.activation(
                    gaint[k][:, lo:hi], Ct[:, lo:hi], AF.Exp, scale=c4w_col
                )

            def apply_out(k, lo, hi):
                n = hi - lo
                od = odt[k]
                ov = bass.AP(od.tensor, od.offset + lo,
                             [list(od[:].ap[0]), [W_CH, C], [1, n]])
                xk = xt[k]
                xv = bass.AP(xk.tensor, xk.offset + lo,
                             [list(xk[:].ap[0]), [W_CH, C], [1, n]])
                g = gaint[k][:, lo:hi]
                gv = bass.AP(g.tensor, g.offset, [list(g.ap[0]), [0, C], [1, n]])
                nc.vector.tensor_tensor(ov, gv, xv, OP.mult)
                q0, q1 = (nc.sync, nc.scalar) if k % 2 == 0 else (nc.scalar, nc.sync)
                for c in range(C):
                    dof = k * CW + c * W_CH + lo
                    sof = c * W_CH + lo
                    q0.dma_start(
                        out=od_ext[0:64, dof : dof + n], in_=od[0:64, sof : sof + n]
                    )
                    q1.dma_start(
                        out=od_ext[64:128, dof : dof + n], in_=od[64:128, sof : sof + n]
                    )

            odt = [
                po.tile([P, CW], BF16, tag="od", name=f"od{k}") for k in range(NCH)
            ]
            for k in range(NCH):
                vt[k] = pv.tile([P, W_CH], F32, tag="v", name=f"v{k}")
                gaint[k] = pg.tile([P, W_CH], BF16, tag=f"g{k}", name=f"g{k}")

            # chunk 0 tail (fix-free) as soon as scan0 done
            nc.scalar.activation(
                vt[0][:, JF:W_CH], y1[0][:, JF:W_CH], AF.Ln,
                bias=eps_col, scale=lns_col,
            )
            knee(0, JF, W_CH, nc.vector)

            # chunks 1..3: ln on ACT, knee on Pool
            for k in range(1, NCH):
                nc.scalar.activation(
                    vt[k][:], y1[k][:], AF.Ln, bias=eps_col, scale=lns_col
                )
                knee(k, 0, W_CH, nc.gpsimd if TS_POOL[k] else nc.vector)

            # ---- cross-partition carry fix (after scan3) ------------------
            s_col = pps.tile([P, 1], F32, tag="s_col")
            nc.tensor.matmul(
                s_col[:], tri_ap, y1[NCH - 1][:, W_CH - 1 : W_CH],
                start=True, stop=True,
            )
            nc.vector.scalar_tensor_tensor(
                y1[0][:, 0:JF], pw_ap, s_col[:, 0:1], y1[0][:, 0:JF],
                OP.mult, OP.add,
            )
            nc.scalar.activation(
                vt[0][:, 0:JF], y1[0][:, 0:JF], AF.Ln,
                bias=eps_col, scale=lns_col,
            )
            knee(0, 0, JF, nc.vector)

            # ---- apply + store: tail chunks first, chunk0 head last ------
            apply_out(0, JF, W_CH)
            for k in range(1, NCH):
                apply_out(k, 0, W_CH)
            apply_out(0, 0, JF)

    nc.finalize()
    return nc


def host_params(z_alpha, log_threshold, log_ratio, log_knee):
    z = z_alpha.astype(np.float64).reshape(-1)
    alpha = 1.0 / (1.0 + np.exp(-z))
    aK = np.exp(K_FIR * np.log(alpha))
    assert np.all(aK < 1e-6), "FIR tail non-negligible; needs shift correction"
    aJ = np.exp(JF * np.log(alpha))
    assert np.all(aJ < 1e-7), "carry-fix reach JF too small for this alpha"
    T = log_threshold.astype(np.float64).reshape(-1) - 6.0
    R = 1.0 + np.exp(log_ratio.astype(np.float64).reshape(-1))
    W = np.exp(log_knee.astype(np.float64).reshape(-1))
    c = 1.0 / R - 1.0
    b1 = W - T  # v = LG + b1
    assert np.all(b1 < 60.0), "ln-fold scale would overflow f32"

    n = alpha.shape[0]
    auxs = []
    j = np.arange(1, JF + 1, dtype=np.float64)
    kq = np.arange(SPP)[None, :] - 1 - np.arange(SPP)[:, None]
    for c0 in range(n // NLOC):
        sl = slice(c0 * NLOC, (c0 + 1) * NLOC)
        a4, T4, W4, c4, b14 = alpha[sl], T[sl], W[sl], c[sl], b1[sl]
        eb = np.exp(b14)
        aux = np.zeros((P, NAUX), np.float64)
        rep = np.repeat
        aux[:, A_ALPHA] = rep(a4, SPP)
        aux[:, A_LNS] = rep(eb * 0.5 * (1.0 - a4), SPP)
        aux[:, A_EPS] = rep(eb * EPS, SPP)
        aux[:, A_W2] = rep(2.0 * W4, SPP)
        aux[:, A_C4W] = rep(c4 / (4.0 * W4), SPP)
        for s in range(NLOC):
            expo = FCH * kq * np.log(a4[s])
            m = (kq >= 0) & (expo > -100.0)
            blk = np.zeros((SPP, SPP))
            blk[m] = np.exp(expo[m])
            aux[s * SPP : (s + 1) * SPP, A_TRI : A_TRI + P][
                :, s * SPP : (s + 1) * SPP
            ] = blk
            aux[s * SPP : (s + 1) * SPP, A_PW : A_PW + JF] = np.exp(
                j * np.log(a4[s])
            )[None, :]
        auxs.append(aux.astype(np.float32))
    return auxs


def shuffle_in(x_core):
    """(NLOC, C, L) f32 -> (P, ROW) bf16 device layout (chunked rows)."""
    xb = x_core.astype(np.float32).astype(ml_dtypes.bfloat16)
    v = xb.reshape(NLOC, C, SPP, NCH, W_CH).transpose(0, 2, 3, 1, 4)
    return np.ascontiguousarray(v.reshape(P, ROW))


def unshuffle_out(od):
    """(P, ROW) bf16 device layout -> (NLOC, C, L) f32."""
    v = od.reshape(NLOC, SPP, NCH, C, W_CH).astype(np.float32)
    return v.transpose(0, 3, 1, 2, 4).reshape(NLOC, C, L)


def _ensure_ntff_hook():
    import types

    try:
        from antenv.axon_hooks import get_axon_ntff_profile_hook  # noqa: F401

        return
    except ImportError:
        pass
    try:
        from trn_agent_boot.trn_boot import _ntff_profile_via_ctypes
    except ImportError:
        return
    hook = _ntff_profile_via_ctypes("/opt/axon/libaxon_pjrt.so")
    mod = types.ModuleType("antenv.axon_hooks")
    mod._hook = hook
    mod.get_axon_ntff_profile_hook = lambda: mod._hook

    def set_axon_ntff_profile_hook(h):
        mod._hook = h

    mod.set_axon_ntff_profile_hook = set_axon_ntff_profile_hook
    import antenv

    sys.modules["antenv.axon_hooks"] = mod
    antenv.axon_hooks = mod


def kernel(input_signals, z_alpha, log_threshold, log_ratio, log_knee):
    from concourse.bass_utils import run_bass_kernel_spmd

    x = np.asarray(input_signals, np.float32)
    auxs = host_params(
        np.asarray(z_alpha), np.asarray(log_threshold),
        np.asarray(log_ratio), np.asarray(log_knee),
    )

    nc = build_nc()
    core_ids = list(range(NCORES))
    in_maps = [
        {"xd": shuffle_in(x[i * NLOC : (i + 1) * NLOC]), "aux": auxs[i]}
        for i in core_ids
    ]

    trace = os.environ.get("BASS_KERNEL_TRACE", "0") == "1"
    if trace:
        _ensure_ntff_hook()
    res = run_bass_kernel_spmd(nc, in_maps, core_ids, trace=trace)
    if trace:
        TRACE_RESULT["exec_time_ns"] = res.exec_time_ns
        TRACE_RESULT["results"] = res

    out = np.empty((N, C, L), np.float32)
    for i in core_ids:
        out[i * NLOC : (i + 1) * NLOC] = unshuffle_out(
            np.asarray(res.results[i]["od"])
        )
    return out


# revision 11
# speedup vs baseline: 1.8426x; 1.8426x over previous
"""Trainium2 Bass kernel for nn_ApproxCompressor (v5).

Reference (per sample n):
    alpha = sigmoid(z_alpha); h[k] = (1-alpha)*alpha^k (k<16384)
    env   = causal_conv(mean_c x^2, h); LG = log(env + 1e-5)
    quadratic-knee gain; out = gain * x.

v5 strategy (8 cores x 4 samples, pure data parallel):
  * Layout: sample s owns partitions [32s, 32s+32); each partition holds a
    contiguous 4096-sample run, stored as 4 chunks of [ch0 1024 | ch1 1024]
    (4KB contiguous per partition per chunk) for chunk-granular DMA.
  * 16K-tap FIR == one-pole IIR via DVE tensor_tensor_scan per chunk,
    chained via `initial`; cross-partition carries fixed by a block-diagonal
    decay matmul (PE) + power-table stt on the first JF cols of chunk 0.
    Chunk 0 is split head[0:JF] (waits on fix) / tail[JF:] (early).
  * Exact knee, 0 relu + 0 stt:  v = ln(e^{W-T}*(s*y + eps)) = LG - T + W
    (shift folded into ln scale/bias);  C = clamp(v,0,2W) and Z' = 2v-2W
    are single dual-op tensor_scalars (DVE 2x_2p); Z = max(Z',C), Q = C*Z
    bf16 tensor_tensors; gain = exp(c4w*Q) via ACT scale.
  * Knee ops for chunks 1..3 run on the idle Pool engine (nc.gpsimd);
    chunk 0 stays on DVE (end-of-kernel slack).
  * Params packed in one aux tensor [128, 8+128+256], loaded via Pool SWDGE.
"""

import os
import sys

import numpy as np


def _import_concourse():
    try:
        import concourse.bass  # noqa: F401
    except ImportError:
        for p in ("/opt/trn_rl_repo", "/root/.axon_site/_ro/trn_rl_repo"):
            if os.path.isdir(p) and p not in sys.path:
                sys.path.insert(0, p)
        import concourse.bass  # noqa: F401


_import_concourse()

import ml_dtypes  # noqa: E402
import concourse.bass as bass  # noqa: E402
import concourse.tile as tile  # noqa: E402
from concourse import bacc, mybir  # noqa: E402

N, C, L = 32, 2, 131072
NCORES = 8
NLOC = N // NCORES  # 4 samples/core
P = 128
SPP = P // NLOC  # 32 partitions/sample
FCH = L // SPP  # 4096 samples per partition row
NCH = 4  # chunks
W_CH = FCH // NCH  # 1024
JF = 256
EPS = 1e-5
K_FIR = 16384
ROW = C * FCH  # 8192 elems per device-layout row
CW = C * W_CH  # 2048 elems per chunk row

F32 = mybir.dt.float32
BF16 = mybir.dt.bfloat16

# aux columns: 0 alpha | 1 lnscale2 | 2 eps2 | 3 w2 | 4 c4w ; 8:136 tri ; 136:392 pw
A_ALPHA, A_LNS, A_EPS, A_W2, A_C4W = 0, 1, 2, 3, 4
A_TRI, A_PW = 8, 136
NAUX = A_PW + JF

ACT_SET_ID = 6
# probe: dual-op tensor_scalars (C, Z') on Pool for chunks 1-3
TS_POOL = (False, False, False, False)
# probe: scan engine per chunk (Pool lowering of TensorScalarPtr-scan)
SCAN_POOL = (False, False, False, False)

TRACE_RESULT = {}


def _bcast(col_ap, n):
    return bass.AP(col_ap.tensor, col_ap.offset, [list(col_ap.ap[0]), [0, n]])


def _view3(ap2, c, w):
    """[p, w] slice -> [p, c, w] with stride-0 mid dim if c>1 over gain."""
    return bass.AP(ap2.tensor, ap2.offset, [list(ap2.ap[0]), [0, c], [1, w]])


def build_nc():
    AF = mybir.ActivationFunctionType
    OP = mybir.AluOpType

    nc = bacc.Bacc("TRN2", target_bir_lowering=False, num_devices=NCORES)
    xd_ext = nc.declare_dram_parameter("xd", [P, ROW], BF16, isOutput=False)
    aux_ext = nc.declare_dram_parameter("aux", [P, NAUX], F32, isOutput=False)
    od_ext = nc.declare_dram_parameter("od", [P, ROW], BF16, isOutput=True)

    with tile.TileContext(nc) as tc:
        atl = mybir.InstLoadActFuncSet(
            name=nc.get_next_instruction_name(), ins=[], outs=[],
            act_func_set_id=ACT_SET_ID,
        )
        nc.scalar.add_instruction(atl)
        with (
            tc.tile_pool(name="pc", bufs=1) as pc,
            tc.tile_pool(name="pin", bufs=NCH) as pin,
            tc.tile_pool(name="po", bufs=NCH) as po,
            tc.tile_pool(name="py", bufs=NCH) as py,
            tc.tile_pool(name="psq", bufs=3) as psq,
            tc.tile_pool(name="pD", bufs=NCH) as pD,
            tc.tile_pool(name="pv", bufs=NCH) as pv,
            tc.tile_pool(name="pcz", bufs=6) as pcz,
            tc.tile_pool(name="pg", bufs=NCH) as pg,
            tc.tile_pool(name="pps", bufs=2, space=bass.MemorySpace.PSUM) as pps,
        ):
            aux = pc.tile([P, NAUX], F32, tag="aux")
            nc.gpsimd.dma_start(out=aux[:], in_=aux_ext[:])

            a_col = aux[:, A_ALPHA : A_ALPHA + 1]
            lns_col = aux[:, A_LNS : A_LNS + 1]
            eps_col = aux[:, A_EPS : A_EPS + 1]
            w2_col = aux[:, A_W2 : A_W2 + 1]
            c4w_col = aux[:, A_C4W : A_C4W + 1]
            tri_ap = aux[:, A_TRI : A_TRI + P]
            pw_ap = aux[:, A_PW : A_PW + JF]

            # ---- input: per-chunk tiles, 2 partition-half DMA calls each ----
            xt = []
            for k in range(NCH):
                xk = pin.tile([P, CW], BF16, tag="xk")
                nc.sync.dma_start(
                    out=xk[0:64, :], in_=xd_ext[0:64, k * CW : (k + 1) * CW]
                )
                nc.scalar.dma_start(
                    out=xk[64:128, :], in_=xd_ext[64:128, k * CW : (k + 1) * CW]
                )
                xt.append(xk)

            # ---- energy (ACT square + DVE add) + chained scans ----------
            y1 = []
            for k in range(NCH):
                sq = psq.tile([P, CW], BF16, tag="sq")
                nc.scalar.activation(sq[:], xt[k][:], AF.Square)
                D = pD.tile([P, W_CH], BF16, tag="D")
                nc.vector.tensor_tensor(
                    D[:], sq[:, 0:W_CH], sq[:, W_CH:CW], OP.add
                )
                yk = py.tile([P, W_CH], F32, tag="y1")
                init = 0.0 if k == 0 else y1[k - 1][:, W_CH - 1 : W_CH]
                seng = nc.gpsimd if SCAN_POOL[k] else nc.vector
                seng.tensor_tensor_scan(
                    yk[:], _bcast(a_col, W_CH), D[:], init, OP.mult, OP.add
                )
                y1.append(yk)

            # ---- knee pipeline -------------------------------------------
            vt = [None] * NCH
            gaint = [None] * NCH

            def knee(k, lo, hi, ts_eng):
                """v[lo:hi] -> gain[lo:hi] for chunk k; ts ops on ts_eng."""
                v = vt[k][:, lo:hi]
                Ct = pcz.tile([P, W_CH], BF16, tag="C", name=f"C{k}")
                Zp = pcz.tile([P, W_CH], BF16, tag="Zp", name=f"Zp{k}")
                ts_eng.tensor_scalar(Ct[:, lo:hi], v, w2_col, 0.0, OP.min, OP.max)
                ts_eng.tensor_scalar(
                    Zp[:, lo:hi], v, 2.0, w2_col, OP.mult, OP.subtract
                )
                nc.vector.tensor_tensor(Zp[:, lo:hi], Zp[:, lo:hi], Ct[:, lo:hi], OP.max)
                nc.vector.tensor_tensor(Ct[:, lo:hi], Ct[:, lo:hi], Zp[:, lo:hi], OP.mult)
                nc.scalar.activation(
                    gaint[k][:, lo:hi], Ct[:, lo:hi], AF.Exp, scale=c4w_col
                )

            def apply_out(k, lo, hi):
                n = hi - lo
                od = odt[k]
                ov = bass.AP(od.tensor, od.offset + lo,
                             [list(od[:].ap[0]), [W_CH, C], [1, n]])
                xk = xt[k]
                xv = bass.AP(xk.tensor, xk.offset + lo,
                             [list(xk[:].ap[0]), [W_CH, C], [1, n]])
                g = gaint[k][:, lo:hi]
                gv = bass.AP(g.tensor, g.offset, [list(g.ap[0]), [0, C], [1, n]])
                nc.vector.tensor_tensor(ov, gv, xv, OP.mult)
                q0, q1 = (nc.sync, nc.scalar) if k % 2 == 0 else (nc.scalar, nc.sync)
                for c in range(C):
                    dof = k * CW + c * W_CH + lo
                    sof = c * W_CH + lo
                    q0.dma_start(
                        out=od_ext[0:64, dof : dof + n], in_=od[0:64, sof : sof + n]
                    )
                    q1.dma_start(
                        out=od_ext[64:128, dof : dof + n], in_=od[64:128, sof : sof + n]
                    )

            odt = [
                po.tile([P, CW], BF16, tag="od", name=f"od{k}") for k in range(NCH)
            ]
            for k in range(NCH):
                vt[k] = pv.tile([P, W_CH], F32, tag="v", name=f"v{k}")
                gaint[k] = pg.tile([P, W_CH], BF16, tag=f"g{k}", name=f"g{k}")

            # chunk 0 tail (fix-free) as soon as scan0 done
            nc.scalar.activation(
                vt[0][:, JF:W_CH], y1[0][:, JF:W_CH], AF.Ln,
                bias=eps_col, scale=lns_col,
            )
            knee(0, JF, W_CH, nc.vector)

            # chunks 1..3: ln on ACT, knee on Pool
            for k in range(1, NCH):
                nc.scalar.activation(
                    vt[k][:], y1[k][:], AF.Ln, bias=eps_col, scale=lns_col
                )
                knee(k, 0, W_CH, nc.gpsimd if TS_POOL[k] else nc.vector)

            # ---- cross-partition carry fix (after scan3) ------------------
            s_col = pps.tile([P, 1], F32, tag="s_col")
            nc.tensor.matmul(
                s_col[:], tri_ap, y1[NCH - 1][:, W_CH - 1 : W_CH],
                start=True, stop=True,
            )
            nc.vector.scalar_tensor_tensor(
                y1[0][:, 0:JF], pw_ap, s_col[:, 0:1], y1[0][:, 0:JF],
                OP.mult, OP.add,
            )
            nc.scalar.activation(
                vt[0][:, 0:JF], y1[0][:, 0:JF], AF.Ln,
                bias=eps_col, scale=lns_col,
            )
            knee(0, 0, JF, nc.vector)

            # ---- apply + store: tail chunks first, chunk0 head last ------
            apply_out(0, JF, W_CH)
            for k in range(1, NCH):
                apply_out(k, 0, W_CH)
            apply_out(0, 0, JF)

    nc.finalize()
    return nc


def host_params(z_alpha, log_threshold, log_ratio, log_knee):
    z = z_alpha.astype(np.float64).reshape(-1)
    alpha = 1.0 / (1.0 + np.exp(-z))
    aK = np.exp(K_FIR * np.log(alpha))
    assert np.all(aK < 1e-6), "FIR tail non-negligible; needs shift correction"
    aJ = np.exp(JF * np.log(alpha))
    assert np.all(aJ < 1e-7), "carry-fix reach JF too small for this alpha"
    T = log_threshold.astype(np.float64).reshape(-1) - 6.0
    R = 1.0 + np.exp(log_ratio.astype(np.float64).reshape(-1))
    W = np.exp(log_knee.astype(np.float64).reshape(-1))
    c = 1.0 / R - 1.0
    b1 = W - T  # v = LG + b1
    assert np.all(b1 < 60.0), "ln-fold scale would overflow f32"

    n = alpha.shape[0]
    auxs = []
    j = np.arange(1, JF + 1, dtype=np.float64)
    kq = np.arange(SPP)[None, :] - 1 - np.arange(SPP)[:, None]
    for c0 in range(n // NLOC):
        sl = slice(c0 * NLOC, (c0 + 1) * NLOC)
        a4, T4, W4, c4, b14 = alpha[sl], T[sl], W[sl], c[sl], b1[sl]
        eb = np.exp(b14)
        aux = np.zeros((P, NAUX), np.float64)
        rep = np.repeat
        aux[:, A_ALPHA] = rep(a4, SPP)
        aux[:, A_LNS] = rep(eb * 0.5 * (1.0 - a4), SPP)
        aux[:, A_EPS] = rep(eb * EPS, SPP)
        aux[:, A_W2] = rep(2.0 * W4, SPP)
        aux[:, A_C4W] = rep(c4 / (4.0 * W4), SPP)
        for s in range(NLOC):
            expo = FCH * kq * np.log(a4[s])
            m = (kq >= 0) & (expo > -100.0)
            blk = np.zeros((SPP, SPP))
            blk[m] = np.exp(expo[m])
            aux[s * SPP : (s + 1) * SPP, A_TRI : A_TRI + P][
                :, s * SPP : (s + 1) * SPP
            ] = blk
            aux[s * SPP : (s + 1) * SPP, A_PW : A_PW + JF] = np.exp(
                j * np.log(a4[s])
            )[None, :]
        auxs.append(aux.astype(np.float32))
    return auxs


def shuffle_in(x_core):
    """(NLOC, C, L) f32 -> (P, ROW) bf16 device layout (chunked rows)."""
    xb = x_core.astype(np.float32).astype(ml_dtypes.bfloat16)
    v = xb.reshape(NLOC, C, SPP, NCH, W_CH).transpose(0, 2, 3, 1, 4)
    return np.ascontiguousarray(v.reshape(P, ROW))


def unshuffle_out(od):
    """(P, ROW) bf16 device layout -> (NLOC, C, L) f32."""
    v = od.reshape(NLOC, SPP, NCH, C, W_CH).astype(np.float32)
    return v.transpose(0, 3, 1, 2, 4).reshape(NLOC, C, L)


def _ensure_ntff_hook():
    import types

    try:
        from antenv.axon_hooks import get_axon_ntff_profile_hook  # noqa: F401

        return
    except ImportError:
        pass
    try:
        from trn_agent_boot.trn_boot import _ntff_profile_via_ctypes
    except ImportError:
        return
    hook = _ntff_profile_via_ctypes("/opt/axon/libaxon_pjrt.so")
    mod = types.ModuleType("antenv.axon_hooks")
    mod._hook = hook
    mod.get_axon_ntff_profile_hook = lambda: mod._hook

    def set_axon_ntff_profile_hook(h):
        mod._hook = h

    mod.set_axon_ntff_profile_hook = set_axon_ntff_profile_hook
    import antenv

    sys.modules["antenv.axon_hooks"] = mod
    antenv.axon_hooks = mod


def kernel(input_signals, z_alpha, log_threshold, log_ratio, log_knee):
    from concourse.bass_utils import run_bass_kernel_spmd

    x = np.asarray(input_signals, np.float32)
    auxs = host_params(
        np.asarray(z_alpha), np.asarray(log_threshold),
        np.asarray(log_ratio), np.asarray(log_knee),
    )

    nc = build_nc()
    core_ids = list(range(NCORES))
    in_maps = [
        {"xd": shuffle_in(x[i * NLOC : (i + 1) * NLOC]), "aux": auxs[i]}
        for i in core_ids
    ]

    trace = os.environ.get("BASS_KERNEL_TRACE", "0") == "1"
    if trace:
        _ensure_ntff_hook()
    res = run_bass_kernel_spmd(nc, in_maps, core_ids, trace=trace)
    if trace:
        TRACE_RESULT["exec_time_ns"] = res.exec_time_ns
        TRACE_RESULT["results"] = res

    out = np.empty((N, C, L), np.float32)
    for i in core_ids:
        out[i * NLOC : (i + 1) * NLOC] = unshuffle_out(
            np.asarray(res.results[i]["od"])
        )
    return out


# revision 15
# speedup vs baseline: 1.8596x; 1.0093x over previous
"""Trainium2 Bass kernel for nn_ApproxCompressor (v5).

Reference (per sample n):
    alpha = sigmoid(z_alpha); h[k] = (1-alpha)*alpha^k (k<16384)
    env   = causal_conv(mean_c x^2, h); LG = log(env + 1e-5)
    quadratic-knee gain; out = gain * x.

v5 strategy (8 cores x 4 samples, pure data parallel):
  * Layout: sample s owns partitions [32s, 32s+32); each partition holds a
    contiguous 4096-sample run, stored as 4 chunks of [ch0 1024 | ch1 1024]
    (4KB contiguous per partition per chunk) for chunk-granular DMA.
  * 16K-tap FIR == one-pole IIR via DVE tensor_tensor_scan per chunk,
    chained via `initial`; cross-partition carries fixed by a block-diagonal
    decay matmul (PE) + power-table stt on the first JF cols of chunk 0.
    Chunk 0 is split head[0:JF] (waits on fix) / tail[JF:] (early).
  * Exact knee, 0 relu + 0 stt:  v = ln(e^{W-T}*(s*y + eps)) = LG - T + W
    (shift folded into ln scale/bias);  C = clamp(v,0,2W) and Z' = 2v-2W
    are single dual-op tensor_scalars (DVE 2x_2p); Z = max(Z',C), Q = C*Z
    bf16 tensor_tensors; gain = exp(c4w*Q) via ACT scale.
  * Knee ops for chunks 1..3 run on the idle Pool engine (nc.gpsimd);
    chunk 0 stays on DVE (end-of-kernel slack).
  * Params packed in one aux tensor [128, 8+128+256], loaded via Pool SWDGE.
"""

import os
import sys

import numpy as np


def _import_concourse():
    try:
        import concourse.bass  # noqa: F401
    except ImportError:
        for p in ("/opt/trn_rl_repo", "/root/.axon_site/_ro/trn_rl_repo"):
            if os.path.isdir(p) and p not in sys.path:
                sys.path.insert(0, p)
        import concourse.bass  # noqa: F401


_import_concourse()

import ml_dtypes  # noqa: E402
import concourse.bass as bass  # noqa: E402
import concourse.tile as tile  # noqa: E402
from concourse import bacc, mybir  # noqa: E402

N, C, L = 32, 2, 131072
NCORES = 8
NLOC = N // NCORES  # 4 samples/core
P = 128
SPP = P // NLOC  # 32 partitions/sample
FCH = L // SPP  # 4096 samples per partition row
NCH = 4  # chunks
W_CH = FCH // NCH  # 1024
JF = 256
EPS = 1e-5
K_FIR = 16384
ROW = C * FCH  # 8192 elems per device-layout row
CW = C * W_CH  # 2048 elems per chunk row

F32 = mybir.dt.float32
BF16 = mybir.dt.bfloat16

# aux columns: 0 alpha | 1 lnscale2 | 2 eps2 | 3 w2 | 4 c4w ; 8:136 tri ; 136:392 pw
A_ALPHA, A_LNS, A_EPS, A_W2, A_C4W = 0, 1, 2, 3, 4
A_TRI, A_PW = 8, 136
NAUX = A_PW + JF

ACT_SET_ID = 6
# probe: dual-op tensor_scalars (C, Z') on Pool for chunks 1-3
TS_POOL = (False, False, False, False)
# probe: scan engine per chunk (Pool lowering of TensorScalarPtr-scan)
SCAN_POOL = (False, False, False, False)

TRACE_RESULT = {}


def _bcast(col_ap, n):
    return bass.AP(col_ap.tensor, col_ap.offset, [list(col_ap.ap[0]), [0, n]])


def _view3(ap2, c, w):
    """[p, w] slice -> [p, c, w] with stride-0 mid dim if c>1 over gain."""
    return bass.AP(ap2.tensor, ap2.offset, [list(ap2.ap[0]), [0, c], [1, w]])


def build_nc():
    AF = mybir.ActivationFunctionType
    OP = mybir.AluOpType

    nc = bacc.Bacc("TRN2", target_bir_lowering=False, num_devices=NCORES)
    xd_ext = nc.declare_dram_parameter("xd", [P, ROW], BF16, isOutput=False)
    aux_ext = nc.declare_dram_parameter("aux", [P, NAUX], F32, isOutput=False)
    od_ext = nc.declare_dram_parameter("od", [P, ROW], BF16, isOutput=True)

    with tile.TileContext(nc) as tc:
        with (
            tc.tile_pool(name="pc", bufs=1) as pc,
            tc.tile_pool(name="pin", bufs=NCH) as pin,
            tc.tile_pool(name="po", bufs=NCH) as po,
            tc.tile_pool(name="py", bufs=NCH) as py,
            tc.tile_pool(name="psq", bufs=3) as psq,
            tc.tile_pool(name="pD", bufs=NCH) as pD,
            tc.tile_pool(name="pv", bufs=NCH) as pv,
            tc.tile_pool(name="pcz", bufs=6) as pcz,
            tc.tile_pool(name="pg", bufs=NCH) as pg,
            tc.tile_pool(name="pps", bufs=2, space=bass.MemorySpace.PSUM) as pps,
        ):
            aux = pc.tile([P, NAUX], F32, tag="aux")
            nc.gpsimd.dma_start(out=aux[:], in_=aux_ext[:])

            a_col = aux[:, A_ALPHA : A_ALPHA + 1]
            lns_col = aux[:, A_LNS : A_LNS + 1]
            eps_col = aux[:, A_EPS : A_EPS + 1]
            w2_col = aux[:, A_W2 : A_W2 + 1]
            c4w_col = aux[:, A_C4W : A_C4W + 1]
            tri_ap = aux[:, A_TRI : A_TRI + P]
            pw_ap = aux[:, A_PW : A_PW + JF]

            # ---- input: per-chunk tiles, 2 partition-half DMA calls each ----
            xt = []
            for k in range(NCH):
                xk = pin.tile([P, CW], BF16, tag="xk")
                nc.sync.dma_start(
                    out=xk[0:64, :], in_=xd_ext[0:64, k * CW : (k + 1) * CW]
                )
                nc.scalar.dma_start(
                    out=xk[64:128, :], in_=xd_ext[64:128, k * CW : (k + 1) * CW]
                )
                xt.append(xk)

            atl = mybir.InstLoadActFuncSet(
                name=nc.get_next_instruction_name(), ins=[], outs=[],
                act_func_set_id=ACT_SET_ID,
            )
            nc.scalar.add_instruction(atl)

            # ---- energy (ACT square + DVE add) + chained scans ----------
            y1 = []
            for k in range(NCH):
                sq = psq.tile([P, CW], BF16, tag="sq")
                nc.scalar.activation(sq[:], xt[k][:], AF.Square)
                D = pD.tile([P, W_CH], BF16, tag="D")
                nc.vector.tensor_tensor(
                    D[:], sq[:, 0:W_CH], sq[:, W_CH:CW], OP.add
                )
                yk = py.tile([P, W_CH], F32, tag="y1")
                init = 0.0 if k == 0 else y1[k - 1][:, W_CH - 1 : W_CH]
                seng = nc.gpsimd if SCAN_POOL[k] else nc.vector
                seng.tensor_tensor_scan(
                    yk[:], _bcast(a_col, W_CH), D[:], init, OP.mult, OP.add
                )
                y1.append(yk)

            # ---- knee pipeline -------------------------------------------
            vt = [None] * NCH
            gaint = [None] * NCH

            def knee(k, lo, hi, ts_eng):
                """v[lo:hi] -> gain[lo:hi] for chunk k; ts ops on ts_eng."""
                v = vt[k][:, lo:hi]
                Ct = pcz.tile([P, W_CH], BF16, tag="C", name=f"C{k}")
                Zp = pcz.tile([P, W_CH], BF16, tag="Zp", name=f"Zp{k}")
                ts_eng.tensor_scalar(Ct[:, lo:hi], v, w2_col, 0.0, OP.min, OP.max)
                ts_eng.tensor_scalar(
                    Zp[:, lo:hi], v, 2.0, w2_col, OP.mult, OP.subtract
                )
                nc.vector.tensor_tensor(Zp[:, lo:hi], Zp[:, lo:hi], Ct[:, lo:hi], OP.max)
                nc.vector.tensor_tensor(Ct[:, lo:hi], Ct[:, lo:hi], Zp[:, lo:hi], OP.mult)
                nc.scalar.activation(
                    gaint[k][:, lo:hi], Ct[:, lo:hi], AF.Exp, scale=c4w_col
                )

            def apply_out(k, lo, hi):
                n = hi - lo
                od = odt[k]
                ov = bass.AP(od.tensor, od.offset + lo,
                             [list(od[:].ap[0]), [W_CH, C], [1, n]])
                xk = xt[k]
                xv = bass.AP(xk.tensor, xk.offset + lo,
                             [list(xk[:].ap[0]), [W_CH, C], [1, n]])
                g = gaint[k][:, lo:hi]
                gv = bass.AP(g.tensor, g.offset, [list(g.ap[0]), [0, C], [1, n]])
                nc.vector.tensor_tensor(ov, gv, xv, OP.mult)
                q0, q1 = (nc.sync, nc.scalar) if k % 2 == 0 else (nc.scalar, nc.sync)
                if lo == 0 and hi == W_CH:
                    # full chunk: one contiguous 4KB run per partition
                    q0.dma_start(
                        out=od_ext[0:64, k * CW : (k + 1) * CW], in_=od[0:64, :]
                    )
                    q1.dma_start(
                        out=od_ext[64:128, k * CW : (k + 1) * CW], in_=od[64:128, :]
                    )
                else:
                    # partial chunk: 3D AP, two runs of n per partition
                    def _ext3(s2):
                        return bass.AP(s2.tensor, s2.offset,
                                       [list(s2.ap[0]), [W_CH, C], [1, n]])
                    for (po, pn, q) in ((0, 64, q0), (64, 64, q1)):
                        src = _ext3(od[po : po + pn, lo : lo + n])
                        dst = _ext3(
                            od_ext[po : po + pn, k * CW + lo : k * CW + lo + n]
                        )
                        q.dma_start(out=dst, in_=src)

            odt = [
                po.tile([P, CW], BF16, tag="od", name=f"od{k}") for k in range(NCH)
            ]
            for k in range(NCH):
                vt[k] = pv.tile([P, W_CH], F32, tag="v", name=f"v{k}")
                gaint[k] = pg.tile([P, W_CH], BF16, tag=f"g{k}", name=f"g{k}")

            # chunk 0 tail (fix-free) as soon as scan0 done
            nc.scalar.activation(
                vt[0][:, JF:W_CH], y1[0][:, JF:W_CH], AF.Ln,
                bias=eps_col, scale=lns_col,
            )
            knee(0, JF, W_CH, nc.vector)

            # chunks 1..3: ln on ACT, knee on Pool
            for k in range(1, NCH):
                nc.scalar.activation(
                    vt[k][:], y1[k][:], AF.Ln, bias=eps_col, scale=lns_col
                )
                knee(k, 0, W_CH, nc.gpsimd if TS_POOL[k] else nc.vector)

            # ---- cross-partition carry fix (after scan3) ------------------
            s_col = pps.tile([P, 1], F32, tag="s_col")
            nc.tensor.matmul(
                s_col[:], tri_ap, y1[NCH - 1][:, W_CH - 1 : W_CH],
                start=True, stop=True,
            )
            nc.vector.scalar_tensor_tensor(
                y1[0][:, 0:JF], pw_ap, s_col[:, 0:1], y1[0][:, 0:JF],
                OP.mult, OP.add,
            )
            nc.scalar.activation(
                vt[0][:, 0:JF], y1[0][:, 0:JF], AF.Ln,
                bias=eps_col, scale=lns_col,
            )
            knee(0, 0, JF, nc.vector)

            # ---- apply + store: tail chunks first, chunk0 head last ------
            apply_out(0, JF, W_CH)
            for k in range(1, NCH):
                apply_out(k, 0, W_CH)
            apply_out(0, 0, JF)

    nc.finalize()
    return nc


def host_params(z_alpha, log_threshold, log_ratio, log_knee):
    z = z_alpha.astype(np.float64).reshape(-1)
    alpha = 1.0 / (1.0 + np.exp(-z))
    aK = np.exp(K_FIR * np.log(alpha))
    assert np.all(aK < 1e-6), "FIR tail non-negligible; needs shift correction"
    aJ = np.exp(JF * np.log(alpha))
    assert np.all(aJ < 1e-7), "carry-fix reach JF too small for this alpha"
    T = log_threshold.astype(np.float64).reshape(-1) - 6.0
    R = 1.0 + np.exp(log_ratio.astype(np.float64).reshape(-1))
    W = np.exp(log_knee.astype(np.float64).reshape(-1))
    c = 1.0 / R - 1.0
    b1 = W - T  # v = LG + b1
    assert np.all(b1 < 60.0), "ln-fold scale would overflow f32"

    n = alpha.shape[0]
    auxs = []
    j = np.arange(1, JF + 1, dtype=np.float64)
    kq = np.arange(SPP)[None, :] - 1 - np.arange(SPP)[:, None]
    for c0 in range(n // NLOC):
        sl = slice(c0 * NLOC, (c0 + 1) * NLOC)
        a4, T4, W4, c4, b14 = alpha[sl], T[sl], W[sl], c[sl], b1[sl]
        eb = np.exp(b14)
        aux = np.zeros((P, NAUX), np.float64)
        rep = np.repeat
        aux[:, A_ALPHA] = rep(a4, SPP)
        aux[:, A_LNS] = rep(eb * 0.5 * (1.0 - a4), SPP)
        aux[:, A_EPS] = rep(eb * EPS, SPP)
        aux[:, A_W2] = rep(2.0 * W4, SPP)
        aux[:, A_C4W] = rep(c4 / (4.0 * W4), SPP)
        for s in range(NLOC):
            expo = FCH * kq * np.log(a4[s])
            m = (kq >= 0) & (expo > -100.0)
            blk = np.zeros((SPP, SPP))
            blk[m] = np.exp(expo[m])
            aux[s * SPP : (s + 1) * SPP, A_TRI : A_TRI + P][
                :, s * SPP : (s + 1) * SPP
            ] = blk
            aux[s * SPP : (s + 1) * SPP, A_PW : A_PW + JF] = np.exp(
                j * np.log(a4[s])
            )[None, :]
        auxs.append(aux.astype(np.float32))
    return auxs


def shuffle_in(x_core):
    """(NLOC, C, L) f32 -> (P, ROW) bf16 device layout (chunked rows)."""
    xb = x_core.astype(np.float32).astype(ml_dtypes.bfloat16)
    v = xb.reshape(NLOC, C, SPP, NCH, W_CH).transpose(0, 2, 3, 1, 4)
    return np.ascontiguousarray(v.reshape(P, ROW))


def unshuffle_out(od):
    """(P, ROW) bf16 device layout -> (NLOC, C, L) f32."""
    v = od.reshape(NLOC, SPP, NCH, C, W_CH).astype(np.float32)
    return v.transpose(0, 3, 1, 2, 4).reshape(NLOC, C, L)


def _ensure_ntff_hook():
    import types

    try:
        from antenv.axon_hooks import get_axon_ntff_profile_hook  # noqa: F401

        return
    except ImportError:
        pass
    try:
        from trn_agent_boot.trn_boot import _ntff_profile_via_ctypes
    except ImportError:
        return
    hook = _ntff_profile_via_ctypes("/opt/axon/libaxon_pjrt.so")
    mod = types.ModuleType("antenv.axon_hooks")
    mod._hook = hook
    mod.get_axon_ntff_profile_hook = lambda: mod._hook

    def set_axon_ntff_profile_hook(h):
        mod._hook = h

    mod.set_axon_ntff_profile_hook = set_axon_ntff_profile_hook
    import antenv

    sys.modules["antenv.axon_hooks"] = mod
    antenv.axon_hooks = mod


def kernel(input_signals, z_alpha, log_threshold, log_ratio, log_knee):
    from concourse.bass_utils import run_bass_kernel_spmd

    x = np.asarray(input_signals, np.float32)
    auxs = host_params(
        np.asarray(z_alpha), np.asarray(log_threshold),
        np.asarray(log_ratio), np.asarray(log_knee),
    )

    nc = build_nc()
    core_ids = list(range(NCORES))
    in_maps = [
        {"xd": shuffle_in(x[i * NLOC : (i + 1) * NLOC]), "aux": auxs[i]}
        for i in core_ids
    ]

    trace = os.environ.get("BASS_KERNEL_TRACE", "0") == "1"
    if trace:
        _ensure_ntff_hook()
    res = run_bass_kernel_spmd(nc, in_maps, core_ids, trace=trace)
    if trace:
        TRACE_RESULT["exec_time_ns"] = res.exec_time_ns
        TRACE_RESULT["results"] = res

    out = np.empty((N, C, L), np.float32)
    for i in core_ids:
        out[i * NLOC : (i + 1) * NLOC] = unshuffle_out(
            np.asarray(res.results[i]["od"])
        )
    return out


# revision 18
# speedup vs baseline: 2.0877x; 1.1226x over previous
"""Trainium2 Bass kernel for nn_ApproxCompressor (v6).

Reference (per sample n):
    alpha = sigmoid(z_alpha); h[k] = (1-alpha)*alpha^k (k<16384)
    env   = causal_conv(mean_c x^2, h); LG = log(env + 1e-5)
    quadratic-knee gain; out = gain * x.

v6 strategy (8 cores x 4 samples, pure data parallel, PE-based IIR):
  * Time-major layout per sample: partition p holds time t = b*128 + p,
    free dim b = 1024 blocks.  The one-pole IIR envelope becomes block
    matmuls on the idle PE: psum[po,b] = sum_j sum_pin T_j[pin,po] D[pin,b-j]
    with T_j[pin,po] = a^(128j+po-pin) (j=0 causal-triangular), j=0..HIST
    accumulated in PSUM.  HIST chosen so a^(128(HIST+1)) < 1e-8 -> exact to
    f32 noise; zero history blocks handled by a zero-padded D tile.
    No scans, no cross-partition carry fix, no barriers.
  * Exact knee:  v = ln(e^(W-T)*(s*y + eps)) = LG - T + W  (shift folded
    into ln scale/bias, read straight from PSUM);  C = clamp(v,0,2W) and
    Z' = 2v-2W are dual-op tensor_scalars (DVE 2x); Z = max(Z',C), Q = C*Z
    bf16 tensor_tensors; gain = exp(c4w*Q) via ACT scale.
  * Per-sample pipeline: in -> x^2 (ch0 ACT / ch1 DVE) -> D -> PE matmuls
    -> ln -> knee -> exp -> gain*x -> out.  ACT/DVE balanced ~14-17us.
  * aux params per sample as replicated columns; decay matrices shipped
    bf16 via Pool SWDGE.
"""

import os
import sys

import numpy as np


def _import_concourse():
    try:
        import concourse.bass  # noqa: F401
    except ImportError:
        for p in ("/opt/trn_rl_repo", "/root/.axon_site/_ro/trn_rl_repo"):
            if os.path.isdir(p) and p not in sys.path:
                sys.path.insert(0, p)
        import concourse.bass  # noqa: F401


_import_concourse()

import ml_dtypes  # noqa: E402
import concourse.bass as bass  # noqa: E402
import concourse.tile as tile  # noqa: E402
from concourse import bacc, mybir  # noqa: E402

N, C, L = 32, 2, 131072
NCORES = 8
NLOC = N // NCORES  # 4 samples/core
P = 128
BL = P  # block length (time-within-block on partitions)
NB = L // BL  # 1024 blocks per sample
HB = NB // 2  # psum bank half: 512 cols
EPS = 1e-5
ROW = NLOC * C * NB  # 8192 elems per device-layout row (4 samples x 2ch x 1024)
SW = C * NB  # 2048 elems per sample per row

F32 = mybir.dt.float32
BF16 = mybir.dt.bfloat16

# aux: per-sample scalar columns (replicated down all partitions)
# col 4*s+{0,1,2,3} = lnscale2, eps2, w2, c4w  for sample s
NAUX = 16
ACT_SET_ID = 6

TRACE_RESULT = {}


def _bcast(col_ap, n):
    return bass.AP(col_ap.tensor, col_ap.offset, [list(col_ap.ap[0]), [0, n]])


def build_nc(hist):
    AF = mybir.ActivationFunctionType
    OP = mybir.AluOpType
    NT = hist + 1  # number of decay matrices per sample

    nc = bacc.Bacc("TRN2", target_bir_lowering=False, num_devices=NCORES)
    xd_ext = nc.declare_dram_parameter("xd", [P, ROW], BF16, isOutput=False)
    aux_ext = nc.declare_dram_parameter("aux", [P, NAUX], F32, isOutput=False)
    trim_ext = nc.declare_dram_parameter(
        "trim", [P, NLOC * NT * P], BF16, isOutput=False
    )
    od_ext = nc.declare_dram_parameter("od", [P, ROW], BF16, isOutput=True)

    with tile.TileContext(nc) as tc:
        with (
            tc.tile_pool(name="pc", bufs=1) as pc,
            tc.tile_pool(name="pin", bufs=NLOC) as pin,
            tc.tile_pool(name="po", bufs=NLOC) as po,
            tc.tile_pool(name="psq", bufs=2) as psq,
            tc.tile_pool(name="pD", bufs=NLOC) as pD,
            tc.tile_pool(name="pv", bufs=2) as pv,
            tc.tile_pool(name="pcz", bufs=4) as pcz,
            tc.tile_pool(name="pg", bufs=2) as pg,
            tc.tile_pool(name="pps", bufs=NLOC, space=bass.MemorySpace.PSUM) as pps,
        ):
            aux = pc.tile([P, NAUX], F32, tag="aux")
            nc.gpsimd.dma_start(out=aux[:], in_=aux_ext[:])
            trim = pc.tile([P, NLOC * NT * P], BF16, tag="trim")
            nc.gpsimd.dma_start(out=trim[:], in_=trim_ext[:])

            def acol(s, i):
                return aux[:, 4 * s + i : 4 * s + i + 1]

            # ---- input: per-sample tiles, 2 partition-half DMA calls -----
            xt = []
            for s in range(NLOC):
                xs = pin.tile([P, SW], BF16, tag="xs", name=f"xs{s}")
                nc.sync.dma_start(
                    out=xs[0:64, :], in_=xd_ext[0:64, s * SW : (s + 1) * SW]
                )
                nc.scalar.dma_start(
                    out=xs[64:128, :], in_=xd_ext[64:128, s * SW : (s + 1) * SW]
                )
                xt.append(xs)

            atl = mybir.InstLoadActFuncSet(
                name=nc.get_next_instruction_name(), ins=[], outs=[],
                act_func_set_id=ACT_SET_ID,
            )
            nc.scalar.add_instruction(atl)

            # D tiles with zero-padded history head
            Dt = []
            for s in range(NLOC):
                Ds = pD.tile([P, hist + NB], BF16, tag="D", name=f"D{s}")
                if hist:
                    nc.gpsimd.memset(Ds[:, 0:hist], 0.0)
                Dt.append(Ds)

            psum = [
                pps.tile([P, NB], F32, tag="ps", name=f"ps{s}") for s in range(NLOC)
            ]

            def energy_mm(s):
                xs = xt[s]
                sq = psq.tile([P, NB], BF16, tag="sq", name=f"sq{s}")
                sq1 = psq.tile([P, NB], BF16, tag="sq1", name=f"sq1_{s}")
                nc.scalar.activation(sq[:], xs[:, 0:NB], AF.Square)
                nc.vector.tensor_tensor(
                    sq1[:], xs[:, NB:SW], xs[:, NB:SW], OP.mult
                )
                nc.vector.tensor_tensor(
                    Dt[s][:, hist : hist + NB], sq[:], sq1[:], OP.add
                )
                for j in range(NT):
                    w = trim[:, (s * NT + j) * P : (s * NT + j + 1) * P]
                    for h in range(2):
                        nc.tensor.matmul(
                            psum[s][:, h * HB : (h + 1) * HB],
                            w,
                            Dt[s][:, hist + h * HB - j : hist + (h + 1) * HB - j],
                            start=(j == 0),
                            stop=(j == NT - 1),
                        )

            def knee_apply(s):
                v = pv.tile([P, NB], F32, tag="v", name=f"v{s}")
                nc.scalar.activation(
                    v[:], psum[s][:], AF.Ln, bias=acol(s, 1), scale=acol(s, 0)
                )
                Ct = pcz.tile([P, NB], BF16, tag="C", name=f"C{s}")
                Zp = pcz.tile([P, NB], BF16, tag="Zp", name=f"Zp{s}")
                nc.vector.tensor_scalar(
                    Ct[:], v[:], acol(s, 2), 0.0, OP.min, OP.max
                )
                nc.vector.tensor_scalar(
                    Zp[:], v[:], 2.0, acol(s, 2), OP.mult, OP.subtract
                )
                nc.vector.tensor_tensor(Zp[:], Zp[:], Ct[:], OP.max)
                nc.vector.tensor_tensor(Ct[:], Ct[:], Zp[:], OP.mult)
                gain = pg.tile([P, NB], BF16, tag="g", name=f"g{s}")
                nc.scalar.activation(gain[:], Ct[:], AF.Exp, scale=acol(s, 3))

                od = po.tile([P, SW], BF16, tag="od", name=f"od{s}")
                ov = bass.AP(od.tensor, od.offset,
                             [list(od[:].ap[0]), [NB, C], [1, NB]])
                xs = xt[s]
                xv = bass.AP(xs.tensor, xs.offset,
                             [list(xs[:].ap[0]), [NB, C], [1, NB]])
                gv = bass.AP(gain.tensor, gain.offset,
                             [list(gain[:].ap[0]), [0, C], [1, NB]])
                nc.vector.tensor_tensor(ov, gv, xv, OP.mult)
                q0, q1 = (nc.sync, nc.scalar) if s % 2 == 0 else (nc.scalar, nc.sync)
                q0.dma_start(
                    out=od_ext[0:64, s * SW : (s + 1) * SW], in_=od[0:64, :]
                )
                q1.dma_start(
                    out=od_ext[64:128, s * SW : (s + 1) * SW], in_=od[64:128, :]
                )

            # ---- pipeline: energy/matmul per sample, knee trails by one --
            energy_mm(0)
            energy_mm(1)
            knee_apply(0)
            energy_mm(2)
            knee_apply(1)
            energy_mm(3)
            knee_apply(2)
            knee_apply(3)

    nc.finalize()
    return nc


def host_params(z_alpha, log_threshold, log_ratio, log_knee):
    z = z_alpha.astype(np.float64).reshape(-1)
    alpha = 1.0 / (1.0 + np.exp(-z))
    T = log_threshold.astype(np.float64).reshape(-1) - 6.0
    R = 1.0 + np.exp(log_ratio.astype(np.float64).reshape(-1))
    W = np.exp(log_knee.astype(np.float64).reshape(-1))
    c = 1.0 / R - 1.0
    b1 = W - T  # v = LG + b1
    assert np.all(b1 < 60.0), "ln-fold scale would overflow f32"

    # history depth: T_j's smallest exponent is 128j-127; include every j
    # with a^(128j-127) > 1e-8, i.e. j <= (R+127)/128 where a^R = 1e-8
    la = np.log(alpha)
    reach = np.log(1e-8) / la
    hist = int(np.max(np.floor((reach + BL - 1.0) / BL)))
    hist = max(hist, 1)
    assert hist <= 6, f"alpha too close to 1: hist={hist}"
    NT = hist + 1

    n = alpha.shape[0]
    auxs, trims = [], []
    pin_i = np.arange(P)[:, None]
    po_i = np.arange(P)[None, :]
    for c0 in range(n // NLOC):
        sl = slice(c0 * NLOC, (c0 + 1) * NLOC)
        a4, c4, W4, b14 = alpha[sl], c[sl], W[sl], b1[sl]
        eb = np.exp(b14)
        aux = np.zeros((P, NAUX), np.float64)
        trim = np.zeros((P, NLOC * NT * P), np.float64)
        for s in range(NLOC):
            aux[:, 4 * s + 0] = eb[s] * 0.5 * (1.0 - a4[s])
            aux[:, 4 * s + 1] = eb[s] * EPS
            aux[:, 4 * s + 2] = 2.0 * W4[s]
            aux[:, 4 * s + 3] = c4[s] / (4.0 * W4[s])
            for j in range(NT):
                expo = (BL * j + po_i - pin_i) * np.log(a4[s])
                m = expo > -60.0
                if j == 0:
                    m &= po_i >= pin_i
                blk = np.zeros((P, P))
                blk[m] = np.exp(expo[m])
                trim[:, (s * NT + j) * P : (s * NT + j + 1) * P] = blk
        auxs.append(aux.astype(np.float32))
        trims.append(trim.astype(np.float32).astype(ml_dtypes.bfloat16))
    return auxs, trims, hist


def shuffle_in(x_core):
    """(NLOC, C, L) f32 -> (P, ROW) bf16 time-major device layout."""
    xb = x_core.astype(np.float32).astype(ml_dtypes.bfloat16)
    v = xb.reshape(NLOC, C, NB, BL).transpose(3, 0, 1, 2)
    return np.ascontiguousarray(v.reshape(P, ROW))


def unshuffle_out(od):
    """(P, ROW) bf16 device layout -> (NLOC, C, L) f32."""
    v = od.reshape(BL, NLOC, C, NB).astype(np.float32)
    return v.transpose(1, 2, 3, 0).reshape(NLOC, C, L)


def _ensure_ntff_hook():
    import types

    try:
        from antenv.axon_hooks import get_axon_ntff_profile_hook  # noqa: F401

        return
    except ImportError:
        pass
    try:
        from trn_agent_boot.trn_boot import _ntff_profile_via_ctypes
    except ImportError:
        return
    hook = _ntff_profile_via_ctypes("/opt/axon/libaxon_pjrt.so")
    mod = types.ModuleType("antenv.axon_hooks")
    mod._hook = hook
    mod.get_axon_ntff_profile_hook = lambda: mod._hook

    def set_axon_ntff_profile_hook(h):
        mod._hook = h

    mod.set_axon_ntff_profile_hook = set_axon_ntff_profile_hook
    import antenv

    sys.modules["antenv.axon_hooks"] = mod
    antenv.axon_hooks = mod


def kernel(input_signals, z_alpha, log_threshold, log_ratio, log_knee):
    from concourse.bass_utils import run_bass_kernel_spmd

    x = np.asarray(input_signals, np.float32)
    auxs, trims, hist = host_params(
        np.asarray(z_alpha), np.asarray(log_threshold),
        np.asarray(log_ratio), np.asarray(log_knee),
    )

    nc = build_nc(hist)
    core_ids = list(range(NCORES))
    in_maps = [
        {
            "xd": shuffle_in(x[i * NLOC : (i + 1) * NLOC]),
            "aux": auxs[i],
            "trim": trims[i],
        }
        for i in core_ids
    ]

    trace = os.environ.get("BASS_KERNEL_TRACE", "0") == "1"
    if trace:
        _ensure_ntff_hook()
    res = run_bass_kernel_spmd(nc, in_maps, core_ids, trace=trace)
    if trace:
        TRACE_RESULT["exec_time_ns"] = res.exec_time_ns
        TRACE_RESULT["results"] = res

    out = np.empty((N, C, L), np.float32)
    for i in core_ids:
        out[i * NLOC : (i + 1) * NLOC] = unshuffle_out(
            np.asarray(res.results[i]["od"])
        )
    return out
